# revision 22
# baseline (speedup 1.0000x reference)
"""Trainium2 Bass kernel for CrossModalAttention (v4, software-pipelined).

Reference computation (per (b, m) of B=4 x M=3):
    Q = x_q @ Wq.T + bq ; K = x_k @ Wk.T + bk ; V = x_v @ Wv.T + bv
    per head h (4 heads of dim 128):
        scores = Q_h @ K_h.T / sqrt(128)      [2048, 2048]
        attn   = softmax(scores, axis=-1)
        out_h  = attn @ V_h                   [2048, 128]

Sharding over 8 cores: 48 (b*m, head) units, 6 per core.
  core c: slot A = bm c      (all 4 heads)
          slot B = bm 8+c//2 (heads {0,1} if c even else {2,3})

v4 design notes:
  - PE runs ONLY matmuls; the two per-unit 128x512 transposes (softmax
    denominator columns, output [d,q]->[q,d]) are single-call xbar DMA
    transposes with 3D out APs, all serialized on the sync HWDGE ring
    (concurrent xbar use from two rings corrupts data - minitest3).
  - x inputs arrive host-pre-transposed; all loads are plain DMAs split
    across the scalar + sync rings.
  - E and the tree-sum accumulators are FLAT 2D tiles; the denominator
    tree is one contiguous gpsimd add (L1 of k-tiles 0:8) in parallel
    with DVE adds (k-tiles 8:16 + merge levels).
  - every projection sub-task's PSUM->SBUF consumer (bias-add / V copy)
    is emitted one injection slot AFTER its matmuls so the DVE FIFO
    never head-of-line blocks on un-run PE work.
  - 24-unit software pipeline: unit u emits scores at step u, attn@V at
    step u+1, tail (transposes, divide+bias, store) at step u+2.
"""

import sys
import os

for _p in ("/root/.axon_site/_ro/trn_rl_repo", "/opt/trn_rl_repo"):
    if os.path.isdir(_p) and _p not in sys.path:
        sys.path.append(_p)

import numpy as np
import ml_dtypes

import concourse.bass as bass
import concourse.tile as tile
from concourse import bacc, mybir
from concourse.bass_utils import run_bass_kernel_spmd

B, M, NTOK, DIM = 4, 3, 2048, 512
H, HD = 4, 128
NBM = B * M  # 12
NCORES = 8
SCALE = 1.0 / float(np.sqrt(HD))

F32 = mybir.dt.float32
BF16 = mybir.dt.bfloat16

TT = NTOK // 128  # 16 k tiles
CT = DIM // 128  # 4 contraction tiles
QCH = 512  # q processed in chunks of 512
NQC = NTOK // QCH  # 4

# Knobs the test harness may flip before calling kernel():
TRACE = False
TRACE_KWARGS = {}
LAST_RESULTS = None

MULT = mybir.AluOpType.mult
ADD = mybir.AluOpType.add
EXP = mybir.ActivationFunctionType.Exp
AXX = mybir.AxisListType.X


def _build_program():
    nc = bacc.Bacc()
    dram = {}
    for s in ("a", "b"):
        D = 512 if s == "a" else 256
        for nm in ("xq", "xk", "xv"):
            dram[f"{nm}_{s}"] = nc.dram_tensor(
                f"{nm}_{s}", [DIM, NTOK], BF16, kind="ExternalInput"
            )
        for nm in ("wq", "wk", "wv"):
            dram[f"{nm}_{s}"] = nc.dram_tensor(
                f"{nm}_{s}", [DIM, D], BF16, kind="ExternalInput"
            )
        for nm in ("bq", "bk"):
            dram[f"{nm}_{s}"] = nc.dram_tensor(
                f"{nm}_{s}", [D], F32, kind="ExternalInput"
            )
        dram[f"bv_{s}"] = nc.dram_tensor(f"bv_{s}", [D], BF16, kind="ExternalInput")
        dram[f"out_{s}"] = nc.dram_tensor(
            f"out_{s}", [NTOK, D], BF16, kind="ExternalOutput"
        )

    with tile.TileContext(nc) as tc:
        with (
            tc.tile_pool(name="wp", bufs=1) as wp,
            tc.tile_pool(name="biasp", bufs=1) as biasp,
            tc.tile_pool(name="xtp", bufs=1) as xtp,
            tc.tile_pool(name="qkvp", bufs=1) as qkvp,
            tc.tile_pool(name="ep", bufs=2) as ep,
            tc.tile_pool(name="accp", bufs=3) as accp,
            tc.tile_pool(name="accvp", bufs=2) as accvp,
            tc.tile_pool(name="accTp", bufs=2) as accTp,
            tc.tile_pool(name="outTp", bufs=2) as outTp,
            tc.tile_pool(name="otqp", bufs=2) as otqp,
            tc.tile_pool(name="otp", bufs=2) as otp,
            tc.tile_pool(name="recp", bufs=2) as recp,
            tc.tile_pool(name="pst", bufs=2, space="PSUM") as pst,
            tc.tile_pool(name="ppv", bufs=2, space="PSUM") as ppv,
        ):
            _emit(nc, dram, wp, biasp, xtp, qkvp, ep, accp, accvp, accTp,
                  outTp, otqp, otp, recp, pst, ppv)

    nc.finalize()
    return nc


def _emit(nc, dram, wp, biasp, xtp, qkvp, ep, accp, accvp, accTp, outTp,
          otqp, otp, recp, pst, ppv):
    # ---------------- prologue loads, split across the two HWDGE rings ----
    # scalar ring: wq, wk, xq_a, xk_a       sync ring: wv, xv_a, biases
    ws = {}

    def load_w(wname, s, eng):
        D = 512 if s == "a" else 256
        w = wp.tile([128, CT, D], BF16, tag=f"{wname}_{s}", name=f"{wname}_{s}")
        eng.dma_start(
            out=w[:, :, :],
            in_=dram[f"{wname}_{s}"][:].rearrange("(c p) d -> p c d", p=128),
        )
        ws[f"{wname}_{s}"] = w

    xts = {}  # (slot, kind, ct) -> tile ; tags shared between slots

    def load_x(s, kind, eng):
        xr = dram[f"x{kind}_{s}"][:].rearrange("(c p) M -> p c M", p=128)
        for ct in range(CT):
            xt = xtp.tile([128, NTOK], BF16, tag=f"x{kind}{ct}", name=f"x{kind}{ct}")
            eng.dma_start(out=xt[:, :], in_=xr[:, ct])
            xts[(s, kind, ct)] = xt

    bqk = {}
    bvb = {}

    def load_biases(s, D, eng):
        nh = D // 128
        t = biasp.tile([128, 2, nh], F32, tag=f"bqk_{s}", name=f"bqk_{s}")
        eng.dma_start(
            out=t[:, 0, :], in_=dram[f"bq_{s}"][:].rearrange("(j p) -> p j", p=128)
        )
        eng.dma_start(
            out=t[:, 1, :], in_=dram[f"bk_{s}"][:].rearrange("(j p) -> p j", p=128)
        )
        bqk[s] = t
        bv = biasp.tile([128, D], BF16, tag=f"bvb_{s}", name=f"bvb_{s}")
        eng.dma_start(
            out=bv[:, :], in_=dram[f"bv_{s}"][:].unsqueeze(0).to_broadcast([128, D])
        )
        bvb[s] = bv

    # scalar ring: wq, xq, wk (Q-projection critical path), then slot-B w
    # sync ring: biases, xk, wv, xv, slot-B wv/biases
    load_w("wq", "a", nc.scalar)
    load_x("a", "q", nc.scalar)
    load_w("wk", "a", nc.scalar)
    load_biases("a", 512, nc.sync)
    load_x("a", "k", nc.sync)
    load_w("wv", "a", nc.sync)
    load_x("a", "v", nc.sync)
    load_w("wq", "b", nc.scalar)
    load_w("wk", "b", nc.scalar)
    load_w("wv", "b", nc.scalar)
    load_biases("b", 256, nc.scalar)

    QT = {}
    KT = {}
    for s, nh in (("a", 4), ("b", 2)):
        for h in range(nh):
            QT[(s, h)] = qkvp.tile([128, NTOK], BF16, tag=f"qt_{s}{h}",
                                   name=f"qt_{s}{h}")
            KT[(s, h)] = qkvp.tile([128, NTOK], BF16, tag=f"kt_{s}{h}",
                                   name=f"kt_{s}{h}")
    V = {"a": qkvp.tile([128, TT, 512], BF16, tag="v_a", name="v_a"),
         "b": qkvp.tile([128, TT, 256], BF16, tag="v_b", name="v_b")}

    # ---- projection sub-tasks: (emit_mms, emit_post) pairs.  The post
    # (PSUM->SBUF bias-add / copy on DVE) runs one injection slot later so
    # the DVE FIFO never blocks on un-run PE matmuls.
    def qk_sub(s, which, wname, dt, qc4):
        kind = "q" if which == 0 else "k"
        box = {}

        def mms():
            ps = ppv.tile([128, 512], F32, tag="psproj", name="psproj")
            for ct in range(CT):
                nc.tensor.matmul(
                    ps[:, :],
                    ws[f"{wname}_{s}"][:, ct, dt * 128 : (dt + 1) * 128],
                    xts[(s, kind, ct)][:, qc4 * 512 : (qc4 + 1) * 512],
                    start=(ct == 0),
                    stop=(ct == CT - 1),
                )
            box["ps"] = ps

        def post():
            dst = QT[(s, dt)] if which == 0 else KT[(s, dt)]
            nc.vector.tensor_scalar_add(
                dst[:, qc4 * 512 : (qc4 + 1) * 512],
                box["ps"][:, :],
                bqk[s][:, which, dt : dt + 1],
            )
        return (mms, post)

    def v_sub(s, D, tt):
        box = {}

        def mms():
            ps = ppv.tile([128, 512], F32, tag="psproj", name="psproj")
            for ct in range(CT):
                nc.tensor.matmul(
                    ps[:, :D],
                    xts[(s, "v", ct)][:, tt * 128 : (tt + 1) * 128],
                    ws[f"wv_{s}"][:, ct, :],
                    start=(ct == 0),
                    stop=(ct == CT - 1),
                )
            box["ps"] = ps

        def post():
            nc.vector.tensor_copy(V[s][:, tt, :], box["ps"][:, :D])
        return (mms, post)

    def run_subs(subs):
        # emit a list of (mms, post) with posts deferred by one sub
        pending = None
        for mms, post in subs:
            mms()
            if pending is not None:
                pending()
            pending = post
        if pending is not None:
            pending()

    # slot A h0 Q/K emitted in prologue (before step 0), posts deferred
    run_subs([qk_sub("a", w, ("wq", "wk")[w], 0, qc4)
              for qc4 in range(NQC) for w in range(2)])

    # injection schedule: step -> list of (mms, post) or plain closures
    inject = {st: [] for st in range(26)}

    def add_qk(step0, s, h):
        subs = [qk_sub(s, w, ("wq", "wk")[w], h, qc4)
                for w in range(2) for qc4 in range(NQC)]
        for i, sub in enumerate(subs):  # 8 subs over 2 steps
            inject[step0 + i // 4].append(sub)

    def add_v(step0, s, D, nsteps):
        subs = [v_sub(s, D, tt) for tt in range(TT)]
        per = (len(subs) + nsteps - 1) // nsteps
        for i, sub in enumerate(subs):
            inject[step0 + i // per].append(sub)


    add_v(0, "a", 512, 1)        # V_a fully in step 0 (AV_0 reads it step 1)
    add_qk(2, "a", 1)            # h1 by end of step 3 (used step 4)
    add_qk(4, "a", 2)            # h2 by end of step 5 (used step 8)
    add_qk(6, "a", 3)            # h3 by end of step 7 (used step 12)
    # slot-B loads are emitted AFTER a step's g-loop ("late") so a DMA
    # issue waiting on an SBUF tag release never head-of-line blocks the
    # issuing engine mid-step.
    late = {1: [("b", "v", nc.sync)],      # xv tags free after step 0
            8: [("b", "q", nc.scalar),     # xq/xk tags free after step 7
                ("b", "k", nc.sync)]}
    add_v(9, "b", 256, 4)        # steps 9-12 (AV_16 runs in step 17)
    add_qk(13, "b", 0)           # by end of step 14 (used step 16)
    add_qk(16, "b", 1)           # by end of step 17 (used step 20)

    # ---------------- the 24-unit software-pipelined stream ---------------
    units = [("a", h, qc) for h in range(4) for qc in range(NQC)] + \
            [("b", h, qc) for h in range(2) for qc in range(NQC)]

    state = {}

    def emit_scores_g(u, g, st_tile):
        s, h, qc = units[u]
        qsl = slice(qc * QCH, (qc + 1) * QCH)
        for j in range(2):
            kt = 2 * g + j
            nc.tensor.matmul(
                st_tile[:, j * QCH : (j + 1) * QCH],
                KT[(s, h)][:, kt * 128 : (kt + 1) * 128],
                QT[(s, h)][:, qsl],
                start=True,
                stop=True,
            )

    def emit_av_g(u, g):
        s, h, qc = units[u]
        stt = state[u]
        for j in range(2):
            kt = 2 * g + j
            nc.tensor.matmul(
                stt["pv"][:, :],
                V[s][:, kt, h * 128 : (h + 1) * 128],
                stt["E"][:, kt * QCH : (kt + 1) * QCH],
                start=(kt == 0),
                stop=(kt == TT - 1),
            )

    NSTEPS = len(units) + 2
    for step in range(NSTEPS):
        u_cur = step if step < len(units) else None
        u_prev = step - 1 if 1 <= step <= len(units) else None
        u_tail = step - 2 if step >= 2 else None

        if u_cur is not None:
            E = ep.tile([128, TT * QCH], BF16, tag="E", name="E")
            state[u_cur] = {"E": E}
        if u_prev is not None:
            pv = ppv.tile([128, QCH], F32, tag="pvav", name="pvav")
            state[u_prev]["pv"] = pv

        inj = list(inject.get(step, ()))
        ninj = len(inj)

        # interleaved PE stream: projections | attn@V(u-1) | scores(u).
        # Projection posts (DVE) are deferred one g-slot behind their MMs.
        pending_posts = []
        for g in range(8):
            lo = (g * ninj) // 8
            hi = ((g + 1) * ninj) // 8
            posts_due, pending_posts = pending_posts, []
            for mms, post in inj[lo:hi]:
                mms()
                if post is not None:
                    pending_posts.append(post)
            for p in posts_due:
                p()
            if u_prev is not None:
                emit_av_g(u_prev, g)
            if u_cur is not None:
                st_tile = pst.tile([128, 2 * QCH], F32, tag="st", name="st")
                emit_scores_g(u_cur, g, st_tile)
                nc.scalar.activation(
                    state[u_cur]["E"][:, g * 1024 : (g + 1) * 1024],
                    st_tile[:, :],
                    EXP,
                    scale=SCALE,
                )
        for p in pending_posts:
            p()
        for (ls, lk, leng) in late.get(step, ()):
            load_x(ls, lk, leng)

        # tail of unit u-2
        if u_tail is not None:
            ts, th, tqc = units[u_tail]
            stt = state[u_tail]
            accT = accTp.tile([128, NQC, 128], BF16, tag="accT", name="accT")
            nc.sync.dma_start_transpose(out=accT[:, :, :], in_=stt["acc"][:, 0:512])
            otq = otqp.tile([128, NQC, 128], BF16, tag="otq", name="otq")
            nc.sync.dma_start_transpose(out=otq[:, :, :], in_=stt["outT"][:, :])
            dcol4 = recp.tile([128, NQC], F32, tag="dcol4", name="dcol4")
            rec4 = recp.tile([128, NQC], F32, tag="rec4", name="rec4")
            nc.vector.reduce_sum(out=dcol4[:, :], in_=accT[:, :, :], axis=AXX)
            nc.vector.reciprocal(rec4[:, :], dcol4[:, :])
            ot = otp.tile([128, NQC, 128], BF16, tag="ot", name="ot")
            for j in range(NQC):
                eng = nc.vector
                eng.scalar_tensor_tensor(
                    out=ot[:, j, :],
                    in0=otq[:, j, :],
                    scalar=rec4[:, j : j + 1],
                    in1=bvb[ts][:, th * 128 : (th + 1) * 128],
                    op0=MULT,
                    op1=ADD,
                )
            nc.sync.dma_start(
                out=dram[f"out_{ts}"][
                    tqc * QCH : (tqc + 1) * QCH, th * 128 : (th + 1) * 128
                ].rearrange("(j p) d -> p j d", p=128),
                in_=ot[:, :, :],
            )
            del state[u_tail]

        # tree-sum of unit u (gpsimd takes the contiguous L1 of k-tiles 0:8;
        # DVE takes k-tiles 8:16 and the merge levels) + cast of pv(u-1)
        if u_cur is not None:
            E = state[u_cur]["E"]
            acc_g = accp.tile([128, 2048], BF16, tag="acc_g", name="acc_g")
            acc_v = accvp.tile([128, 2048], BF16, tag="acc_v", name="acc_v")
            nc.gpsimd.tensor_add(acc_g[:, :], E[:, 0:2048], E[:, 2048:4096])
            nc.vector.tensor_add(acc_v[:, :], E[:, 4096:6144], E[:, 6144:8192])
            nc.vector.tensor_add(
                acc_v[:, 0:1024], acc_v[:, 0:1024], acc_v[:, 1024:2048]
            )
            if u_prev is not None:
                outT = outTp.tile([128, QCH], BF16, tag="outT", name="outT")
                nc.vector.tensor_copy(outT[:, :], state[u_prev]["pv"][:, :])
                state[u_prev]["outT"] = outT
            nc.gpsimd.tensor_add(
                acc_g[:, 0:1024], acc_g[:, 0:1024], acc_g[:, 1024:2048]
            )
            nc.vector.tensor_add(
                acc_v[:, 0:512], acc_v[:, 0:512], acc_v[:, 512:1024]
            )
            nc.vector.tensor_add(
                acc_g[:, 0:512], acc_g[:, 0:512], acc_g[:, 512:1024]
            )
            nc.vector.tensor_add(
                acc_g[:, 0:512], acc_g[:, 0:512], acc_v[:, 0:512]
            )
            state[u_cur]["acc"] = acc_g
        elif u_prev is not None:
            outT = outTp.tile([128, QCH], BF16, tag="outT", name="outT")
            nc.vector.tensor_copy(outT[:, :], state[u_prev]["pv"][:, :])
            state[u_prev]["outT"] = outT


_PROGRAM = None


def _get_program():
    global _PROGRAM
    if _PROGRAM is None:
        _PROGRAM = _build_program()
    return _PROGRAM


def kernel(query, key, value, Wq, bq, Wk, bk, Wv, bv):
    global LAST_RESULTS
    bf = ml_dtypes.bfloat16
    q = np.ascontiguousarray(
        np.asarray(query, np.float32).reshape(NBM, NTOK, DIM).transpose(0, 2, 1)
    ).astype(bf)
    k = np.ascontiguousarray(
        np.asarray(key, np.float32).reshape(NBM, NTOK, DIM).transpose(0, 2, 1)
    ).astype(bf)
    v = np.ascontiguousarray(
        np.asarray(value, np.float32).reshape(NBM, NTOK, DIM).transpose(0, 2, 1)
    ).astype(bf)
    WqT = np.ascontiguousarray(np.asarray(Wq, np.float32).T).astype(bf)
    WkT = np.ascontiguousarray(np.asarray(Wk, np.float32).T).astype(bf)
    WvT = np.ascontiguousarray(np.asarray(Wv, np.float32).T).astype(bf)
    bq = np.asarray(bq, np.float32)
    bk = np.asarray(bk, np.float32)
    bvb = np.asarray(bv, np.float32).astype(bf)

    in_maps = []
    for c in range(NCORES):
        bm_a = c
        bm_b = 8 + c // 2
        hs = (c % 2) * 256  # head-pair column offset for slot B
        in_maps.append(
            {
                "xq_a": q[bm_a], "xk_a": k[bm_a], "xv_a": v[bm_a],
                "xq_b": q[bm_b], "xk_b": k[bm_b], "xv_b": v[bm_b],
                "wq_a": WqT, "wk_a": WkT, "wv_a": WvT,
                "bq_a": bq, "bk_a": bk, "bv_a": bvb,
                "wq_b": np.ascontiguousarray(WqT[:, hs : hs + 256]),
                "wk_b": np.ascontiguousarray(WkT[:, hs : hs + 256]),
                "wv_b": np.ascontiguousarray(WvT[:, hs : hs + 256]),
                "bq_b": np.ascontiguousarray(bq[hs : hs + 256]),
                "bk_b": np.ascontiguousarray(bk[hs : hs + 256]),
                "bv_b": np.ascontiguousarray(bvb[hs : hs + 256]),
            }
        )

    nc = _get_program()
    res = run_bass_kernel_spmd(
        nc, in_maps, list(range(NCORES)), trace=TRACE, **TRACE_KWARGS
    )
    LAST_RESULTS = res

    out = np.empty((NBM, NTOK, DIM), np.float32)
    for c in range(NCORES):
        hs = (c % 2) * 256
        out[c] = np.asarray(res.results[c]["out_a"], np.float32)
        out[8 + c // 2][:, hs : hs + 256] = np.asarray(
            res.results[c]["out_b"], np.float32
        )
    return out.reshape(B, M, NTOK, DIM)


# revision 25
# speedup vs baseline: 1.0051x; 1.0051x over previous
"""Trainium2 Bass kernel for CrossModalAttention (v4, software-pipelined).

Reference computation (per (b, m) of B=4 x M=3):
    Q = x_q @ Wq.T + bq ; K = x_k @ Wk.T + bk ; V = x_v @ Wv.T + bv
    per head h (4 heads of dim 128):
        scores = Q_h @ K_h.T / sqrt(128)      [2048, 2048]
        attn   = softmax(scores, axis=-1)
        out_h  = attn @ V_h                   [2048, 128]

Sharding over 8 cores: 48 (b*m, head) units, 6 per core.
  core c: slot A = bm c      (all 4 heads)
          slot B = bm 8+c//2 (heads {0,1} if c even else {2,3})

v4 design notes:
  - PE runs ONLY matmuls; the two per-unit 128x512 transposes (softmax
    denominator columns, output [d,q]->[q,d]) are single-call xbar DMA
    transposes with 3D out APs, all serialized on the sync HWDGE ring
    (concurrent xbar use from two rings corrupts data - minitest3).
  - x inputs arrive host-pre-transposed; all loads are plain DMAs split
    across the scalar + sync rings.
  - E and the tree-sum accumulators are FLAT 2D tiles; the denominator
    tree is one contiguous gpsimd add (L1 of k-tiles 0:8) in parallel
    with DVE adds (k-tiles 8:16 + merge levels).
  - every projection sub-task's PSUM->SBUF consumer (bias-add / V copy)
    is emitted one injection slot AFTER its matmuls so the DVE FIFO
    never head-of-line blocks on un-run PE work.
  - 24-unit software pipeline: unit u emits scores at step u, attn@V at
    step u+1, tail (transposes, divide+bias, store) at step u+2.
"""

import sys
import os

for _p in ("/root/.axon_site/_ro/trn_rl_repo", "/opt/trn_rl_repo"):
    if os.path.isdir(_p) and _p not in sys.path:
        sys.path.append(_p)

import numpy as np
import ml_dtypes

import concourse.bass as bass
import concourse.tile as tile
from concourse import bacc, mybir
from concourse.bass_utils import run_bass_kernel_spmd

B, M, NTOK, DIM = 4, 3, 2048, 512
H, HD = 4, 128
NBM = B * M  # 12
NCORES = 8
SCALE = 1.0 / float(np.sqrt(HD))

F32 = mybir.dt.float32
BF16 = mybir.dt.bfloat16

TT = NTOK // 128  # 16 k tiles
CT = DIM // 128  # 4 contraction tiles
QCH = 512  # q processed in chunks of 512
NQC = NTOK // QCH  # 4

# Knobs the test harness may flip before calling kernel():
TRACE = False
TRACE_KWARGS = {}
LAST_RESULTS = None

MULT = mybir.AluOpType.mult
ADD = mybir.AluOpType.add
EXP = mybir.ActivationFunctionType.Exp
AXX = mybir.AxisListType.X


def _build_program():
    nc = bacc.Bacc()
    dram = {}
    for s in ("a", "b"):
        D = 512 if s == "a" else 256
        for nm in ("xq", "xk", "xv"):
            dram[f"{nm}_{s}"] = nc.dram_tensor(
                f"{nm}_{s}", [DIM, NTOK], BF16, kind="ExternalInput"
            )
        for nm in ("wq", "wk", "wv"):
            dram[f"{nm}_{s}"] = nc.dram_tensor(
                f"{nm}_{s}", [DIM, D], BF16, kind="ExternalInput"
            )
        for nm in ("bq", "bk"):
            dram[f"{nm}_{s}"] = nc.dram_tensor(
                f"{nm}_{s}", [D], F32, kind="ExternalInput"
            )
        dram[f"bv_{s}"] = nc.dram_tensor(f"bv_{s}", [D], BF16, kind="ExternalInput")
        dram[f"out_{s}"] = nc.dram_tensor(
            f"out_{s}", [NTOK, D], BF16, kind="ExternalOutput"
        )

    with tile.TileContext(nc) as tc:
        with (
            tc.tile_pool(name="wp", bufs=1) as wp,
            tc.tile_pool(name="biasp", bufs=1) as biasp,
            tc.tile_pool(name="xtp", bufs=1) as xtp,
            tc.tile_pool(name="qkvp", bufs=1) as qkvp,
            tc.tile_pool(name="ep", bufs=2) as ep,
            tc.tile_pool(name="accp", bufs=3) as accp,
            tc.tile_pool(name="accvp", bufs=2) as accvp,
            tc.tile_pool(name="accTp", bufs=2) as accTp,
            tc.tile_pool(name="outTp", bufs=2) as outTp,
            tc.tile_pool(name="otqp", bufs=2) as otqp,
            tc.tile_pool(name="otp", bufs=2) as otp,
            tc.tile_pool(name="recp", bufs=2) as recp,
            tc.tile_pool(name="pst", bufs=2, space="PSUM") as pst,
            tc.tile_pool(name="ppv", bufs=2, space="PSUM") as ppv,
        ):
            _emit(nc, dram, wp, biasp, xtp, qkvp, ep, accp, accvp, accTp,
                  outTp, otqp, otp, recp, pst, ppv)

    nc.finalize()
    return nc


def _emit(nc, dram, wp, biasp, xtp, qkvp, ep, accp, accvp, accTp, outTp,
          otqp, otp, recp, pst, ppv):
    # ---------------- prologue loads, split across the two HWDGE rings ----
    # scalar ring: wq, wk, xq_a, xk_a       sync ring: wv, xv_a, biases
    ws = {}

    def load_w(wname, s, eng):
        D = 512 if s == "a" else 256
        w = wp.tile([128, CT, D], BF16, tag=f"{wname}_{s}", name=f"{wname}_{s}")
        eng.dma_start(
            out=w[:, :, :],
            in_=dram[f"{wname}_{s}"][:].rearrange("(c p) d -> p c d", p=128),
        )
        ws[f"{wname}_{s}"] = w

    xts = {}  # (slot, kind, ct) -> tile ; tags shared between slots

    def load_x(s, kind, eng):
        xr = dram[f"x{kind}_{s}"][:].rearrange("(c p) M -> p c M", p=128)
        for ct in range(CT):
            xt = xtp.tile([128, NTOK], BF16, tag=f"x{kind}{ct}", name=f"x{kind}{ct}")
            eng.dma_start(out=xt[:, :], in_=xr[:, ct])
            xts[(s, kind, ct)] = xt

    bqk = {}
    bvb = {}

    def load_biases(s, D, eng):
        nh = D // 128
        t = biasp.tile([128, 2, nh], F32, tag=f"bqk_{s}", name=f"bqk_{s}")
        eng.dma_start(
            out=t[:, 0, :], in_=dram[f"bq_{s}"][:].rearrange("(j p) -> p j", p=128)
        )
        eng.dma_start(
            out=t[:, 1, :], in_=dram[f"bk_{s}"][:].rearrange("(j p) -> p j", p=128)
        )
        bqk[s] = t
        bv = biasp.tile([128, D], BF16, tag=f"bvb_{s}", name=f"bvb_{s}")
        eng.dma_start(
            out=bv[:, :], in_=dram[f"bv_{s}"][:].unsqueeze(0).to_broadcast([128, D])
        )
        bvb[s] = bv

    # scalar ring: wq, xq, wk (Q-projection critical path), then slot-B w
    # sync ring: biases, xk, wv, xv, slot-B wv/biases
    load_w("wq", "a", nc.scalar)
    load_x("a", "q", nc.scalar)
    load_w("wk", "a", nc.scalar)
    load_biases("a", 512, nc.sync)
    load_x("a", "k", nc.sync)
    load_w("wv", "a", nc.sync)
    load_x("a", "v", nc.sync)
    load_w("wq", "b", nc.scalar)
    load_w("wk", "b", nc.scalar)
    load_w("wv", "b", nc.scalar)
    load_biases("b", 256, nc.scalar)

    QT = {}
    KT = {}
    for s, nh in (("a", 4), ("b", 2)):
        for h in range(nh):
            QT[(s, h)] = qkvp.tile([128, NTOK], BF16, tag=f"qt_{s}{h}",
                                   name=f"qt_{s}{h}")
            KT[(s, h)] = qkvp.tile([128, NTOK], BF16, tag=f"kt_{s}{h}",
                                   name=f"kt_{s}{h}")
    V = {"a": qkvp.tile([128, TT, 512], BF16, tag="v_a", name="v_a"),
         "b": qkvp.tile([128, TT, 256], BF16, tag="v_b", name="v_b")}

    # ---- projection sub-tasks: (emit_mms, emit_post) pairs.  The post
    # (PSUM->SBUF bias-add / copy on DVE) runs one injection slot later so
    # the DVE FIFO never blocks on un-run PE matmuls.
    def qk_sub(s, which, wname, dt, qc4):
        kind = "q" if which == 0 else "k"
        box = {}

        def mms():
            ps = ppv.tile([128, 512], F32, tag="psproj", name="psproj")
            for ct in range(CT):
                nc.tensor.matmul(
                    ps[:, :],
                    ws[f"{wname}_{s}"][:, ct, dt * 128 : (dt + 1) * 128],
                    xts[(s, kind, ct)][:, qc4 * 512 : (qc4 + 1) * 512],
                    start=(ct == 0),
                    stop=(ct == CT - 1),
                )
            box["ps"] = ps

        def post():
            dst = QT[(s, dt)] if which == 0 else KT[(s, dt)]
            nc.vector.tensor_scalar_add(
                dst[:, qc4 * 512 : (qc4 + 1) * 512],
                box["ps"][:, :],
                bqk[s][:, which, dt : dt + 1],
            )
        return (mms, post)

    def v_sub(s, D, tt):
        box = {}

        def mms():
            ps = ppv.tile([128, 512], F32, tag="psproj", name="psproj")
            for ct in range(CT):
                nc.tensor.matmul(
                    ps[:, :D],
                    xts[(s, "v", ct)][:, tt * 128 : (tt + 1) * 128],
                    ws[f"wv_{s}"][:, ct, :],
                    start=(ct == 0),
                    stop=(ct == CT - 1),
                )
            box["ps"] = ps

        def post():
            nc.vector.tensor_copy(V[s][:, tt, :], box["ps"][:, :D])
        return (mms, post)

    def run_subs(subs):
        # emit a list of (mms, post) with posts deferred by one sub
        pending = None
        for mms, post in subs:
            mms()
            if pending is not None:
                pending()
            pending = post
        if pending is not None:
            pending()

    # slot A h0 Q/K emitted in prologue (before step 0), posts deferred
    run_subs([qk_sub("a", w, ("wq", "wk")[w], 0, qc4)
              for qc4 in range(NQC) for w in range(2)])

    # injection schedule: step -> list of (mms, post) or plain closures
    inject = {st: [] for st in range(26)}

    def add_qk(step0, s, h):
        subs = [qk_sub(s, w, ("wq", "wk")[w], h, qc4)
                for w in range(2) for qc4 in range(NQC)]
        for i, sub in enumerate(subs):  # 8 subs over 2 steps
            inject[step0 + i // 4].append(sub)

    def add_v(step0, s, D, nsteps):
        subs = [v_sub(s, D, tt) for tt in range(TT)]
        per = (len(subs) + nsteps - 1) // nsteps
        for i, sub in enumerate(subs):
            inject[step0 + i // per].append(sub)


    # V_a: 12 token-tiles in step 0, last 4 early in step 1 (their posts
    # land at g-slots 1..7, always ahead of AV_0's matching k-tile reads)
    va_subs = [v_sub("a", 512, tt) for tt in range(TT)]
    for sub in va_subs[0:12]:
        inject[0].append(sub)
    for sub in va_subs[12:16]:
        inject[1].append(sub)
    add_qk(2, "a", 1)            # h1 by end of step 3 (used step 4)
    add_qk(4, "a", 2)            # h2 by end of step 5 (used step 8)
    add_qk(6, "a", 3)            # h3 by end of step 7 (used step 12)
    # slot-B loads are emitted AFTER a step's g-loop ("late") so a DMA
    # issue waiting on an SBUF tag release never head-of-line blocks the
    # issuing engine mid-step.
    late = {1: [("b", "v", nc.sync)],      # xv tags free after step 0
            8: [("b", "q", nc.scalar),     # xq/xk tags free after step 7
                ("b", "k", nc.sync)]}
    add_v(9, "b", 256, 4)        # steps 9-12 (AV_16 runs in step 17)
    add_qk(13, "b", 0)           # by end of step 14 (used step 16)
    add_qk(16, "b", 1)           # by end of step 17 (used step 20)

    # ---------------- the 24-unit software-pipelined stream ---------------
    units = [("a", h, qc) for h in range(4) for qc in range(NQC)] + \
            [("b", h, qc) for h in range(2) for qc in range(NQC)]

    state = {}

    def emit_scores_g(u, g, st_tile):
        s, h, qc = units[u]
        qsl = slice(qc * QCH, (qc + 1) * QCH)
        for j in range(2):
            kt = 2 * g + j
            nc.tensor.matmul(
                st_tile[:, j * QCH : (j + 1) * QCH],
                KT[(s, h)][:, kt * 128 : (kt + 1) * 128],
                QT[(s, h)][:, qsl],
                start=True,
                stop=True,
            )

    def emit_av_g(u, g):
        s, h, qc = units[u]
        stt = state[u]
        for j in range(2):
            kt = 2 * g + j
            nc.tensor.matmul(
                stt["pv"][:, :],
                V[s][:, kt, h * 128 : (h + 1) * 128],
                stt["E"][:, kt * QCH : (kt + 1) * QCH],
                start=(kt == 0),
                stop=(kt == TT - 1),
            )

    NSTEPS = len(units) + 2
    for step in range(NSTEPS):
        u_cur = step if step < len(units) else None
        u_prev = step - 1 if 1 <= step <= len(units) else None
        u_tail = step - 2 if step >= 2 else None

        if u_cur is not None:
            E = ep.tile([128, TT * QCH], BF16, tag="E", name="E")
            state[u_cur] = {"E": E}
        if u_prev is not None:
            pv = ppv.tile([128, QCH], F32, tag="pvav", name="pvav")
            state[u_prev]["pv"] = pv

        inj = list(inject.get(step, ()))
        ninj = len(inj)
        # DVE halves of unit u-1's tree-sum run interleaved in THIS step's
        # g-slots: their E input completed last step, so the DVE FIFO never
        # blocks on them, and they no longer delay this step's proj posts.
        tree_parts = state[u_prev].pop("dve_tree", []) if u_prev is not None else []

        # interleaved PE stream: projections | attn@V(u-1) | scores(u).
        # Projection posts (DVE) are deferred one g-slot behind their MMs.
        pending_posts = []
        per_slot = -(-ninj // 8)  # ceil: front-load so posts land early
        for g in range(8):
            lo = min(g * per_slot, ninj)
            hi = min((g + 1) * per_slot, ninj)
            posts_due, pending_posts = pending_posts, []
            for mms, post in inj[lo:hi]:
                mms()
                if post is not None:
                    pending_posts.append(post)
            for p in posts_due:
                p()
            if 2 <= g < 2 + len(tree_parts):
                tree_parts[g - 2]()
            if u_prev is not None:
                emit_av_g(u_prev, g)
            if u_cur is not None:
                st_tile = pst.tile([128, 2 * QCH], F32, tag="st", name="st")
                emit_scores_g(u_cur, g, st_tile)
                nc.scalar.activation(
                    state[u_cur]["E"][:, g * 1024 : (g + 1) * 1024],
                    st_tile[:, :],
                    EXP,
                    scale=SCALE,
                )
        for p in pending_posts:
            p()
        for (ls, lk, leng) in late.get(step, ()):
            load_x(ls, lk, leng)

        # tail of unit u-2
        if u_tail is not None:
            ts, th, tqc = units[u_tail]
            stt = state[u_tail]
            accT = accTp.tile([128, NQC, 128], BF16, tag="accT", name="accT")
            nc.sync.dma_start_transpose(out=accT[:, :, :], in_=stt["acc"][:, 0:512])
            otq = otqp.tile([128, NQC, 128], BF16, tag="otq", name="otq")
            nc.sync.dma_start_transpose(out=otq[:, :, :], in_=stt["outT"][:, :])
            dcol4 = recp.tile([128, NQC], F32, tag="dcol4", name="dcol4")
            rec4 = recp.tile([128, NQC], BF16, tag="rec4", name="rec4")
            nc.vector.reduce_sum(out=dcol4[:, :], in_=accT[:, :, :], axis=AXX)
            with nc.allow_low_precision(reason="softmax denom reciprocal in bf16"):
                nc.vector.reciprocal(rec4[:, :], dcol4[:, :])
            ot = otp.tile([128, NQC, 128], BF16, tag="ot", name="ot")
            for j in range(NQC):
                eng = nc.vector
                eng.scalar_tensor_tensor(
                    out=ot[:, j, :],
                    in0=otq[:, j, :],
                    scalar=rec4[:, j : j + 1],
                    in1=bvb[ts][:, th * 128 : (th + 1) * 128],
                    op0=MULT,
                    op1=ADD,
                )
            nc.sync.dma_start(
                out=dram[f"out_{ts}"][
                    tqc * QCH : (tqc + 1) * QCH, th * 128 : (th + 1) * 128
                ].rearrange("(j p) d -> p j d", p=128),
                in_=ot[:, :, :],
            )
            del state[u_tail]

        # cast of pv(u-1) -> bf16 SBUF for the xbar transpose next step
        if u_prev is not None:
            outT = outTp.tile([128, QCH], BF16, tag="outT", name="outT")
            nc.vector.tensor_copy(outT[:, :], state[u_prev]["pv"][:, :])
            state[u_prev]["outT"] = outT

        # tree-sum of unit u: gpsimd's contiguous halves (k-tiles 0:8 L1 +
        # merge) are emitted now (gpsimd has nothing else queued); the DVE
        # halves become closures run in step u+1's g-slot interleave.
        if u_cur is not None:
            E = state[u_cur]["E"]
            acc_g = accp.tile([128, 2048], BF16, tag="acc_g", name="acc_g")
            nc.gpsimd.tensor_add(acc_g[:, :], E[:, 0:2048], E[:, 2048:4096])
            nc.gpsimd.tensor_add(
                acc_g[:, 0:1024], acc_g[:, 0:1024], acc_g[:, 1024:2048]
            )
            box = {}

            def t_l1v(E=E, box=box):
                acc_v = accvp.tile([128, 2048], BF16, tag="acc_v", name="acc_v")
                nc.vector.tensor_add(
                    acc_v[:, :], E[:, 4096:6144], E[:, 6144:8192]
                )
                box["acc_v"] = acc_v

            def t_l2b(box=box):
                av = box["acc_v"]
                nc.vector.tensor_add(av[:, 0:1024], av[:, 0:1024], av[:, 1024:2048])

            def t_l3v(box=box):
                av = box["acc_v"]
                nc.vector.tensor_add(av[:, 0:512], av[:, 0:512], av[:, 512:1024])

            def t_l3g(acc_g=acc_g):
                nc.vector.tensor_add(
                    acc_g[:, 0:512], acc_g[:, 0:512], acc_g[:, 512:1024]
                )

            def t_l4(acc_g=acc_g, box=box):
                nc.vector.tensor_add(
                    acc_g[:, 0:512], acc_g[:, 0:512], box["acc_v"][:, 0:512]
                )

            state[u_cur]["dve_tree"] = [t_l1v, t_l2b, t_l3v, t_l3g, t_l4]
            state[u_cur]["acc"] = acc_g


_PROGRAM = None


def _get_program():
    global _PROGRAM
    if _PROGRAM is None:
        _PROGRAM = _build_program()
    return _PROGRAM


def kernel(query, key, value, Wq, bq, Wk, bk, Wv, bv):
    global LAST_RESULTS
    bf = ml_dtypes.bfloat16
    q = np.ascontiguousarray(
        np.asarray(query, np.float32).reshape(NBM, NTOK, DIM).transpose(0, 2, 1)
    ).astype(bf)
    k = np.ascontiguousarray(
        np.asarray(key, np.float32).reshape(NBM, NTOK, DIM).transpose(0, 2, 1)
    ).astype(bf)
    v = np.ascontiguousarray(
        np.asarray(value, np.float32).reshape(NBM, NTOK, DIM).transpose(0, 2, 1)
    ).astype(bf)
    WqT = np.ascontiguousarray(np.asarray(Wq, np.float32).T).astype(bf)
    WkT = np.ascontiguousarray(np.asarray(Wk, np.float32).T).astype(bf)
    WvT = np.ascontiguousarray(np.asarray(Wv, np.float32).T).astype(bf)
    bq = np.asarray(bq, np.float32)
    bk = np.asarray(bk, np.float32)
    bvb = np.asarray(bv, np.float32).astype(bf)

    in_maps = []
    for c in range(NCORES):
        bm_a = c
        bm_b = 8 + c // 2
        hs = (c % 2) * 256  # head-pair column offset for slot B
        in_maps.append(
            {
                "xq_a": q[bm_a], "xk_a": k[bm_a], "xv_a": v[bm_a],
                "xq_b": q[bm_b], "xk_b": k[bm_b], "xv_b": v[bm_b],
                "wq_a": WqT, "wk_a": WkT, "wv_a": WvT,
                "bq_a": bq, "bk_a": bk, "bv_a": bvb,
                "wq_b": np.ascontiguousarray(WqT[:, hs : hs + 256]),
                "wk_b": np.ascontiguousarray(WkT[:, hs : hs + 256]),
                "wv_b": np.ascontiguousarray(WvT[:, hs : hs + 256]),
                "bq_b": np.ascontiguousarray(bq[hs : hs + 256]),
                "bk_b": np.ascontiguousarray(bk[hs : hs + 256]),
                "bv_b": np.ascontiguousarray(bvb[hs : hs + 256]),
            }
        )

    nc = _get_program()
    res = run_bass_kernel_spmd(
        nc, in_maps, list(range(NCORES)), trace=TRACE, **TRACE_KWARGS
    )
    LAST_RESULTS = res

    out = np.empty((NBM, NTOK, DIM), np.float32)
    for c in range(NCORES):
        hs = (c % 2) * 256
        out[c] = np.asarray(res.results[c]["out_a"], np.float32)
        out[8 + c // 2][:, hs : hs + 256] = np.asarray(
            res.results[c]["out_b"], np.float32
        )
    return out.reshape(B, M, NTOK, DIM)


# revision 26
# speedup vs baseline: 1.4109x; 1.4038x over previous
"""Trainium2 Bass kernel for CrossModalAttention (v4, software-pipelined).

Reference computation (per (b, m) of B=4 x M=3):
    Q = x_q @ Wq.T + bq ; K = x_k @ Wk.T + bk ; V = x_v @ Wv.T + bv
    per head h (4 heads of dim 128):
        scores = Q_h @ K_h.T / sqrt(128)      [2048, 2048]
        attn   = softmax(scores, axis=-1)
        out_h  = attn @ V_h                   [2048, 128]

Sharding over 8 cores: 48 (b*m, head) units, 6 per core.
  core c: slot A = bm c      (all 4 heads)
          slot B = bm 8+c//2 (heads {0,1} if c even else {2,3})

v4 design notes:
  - PE runs ONLY matmuls; the two per-unit 128x512 transposes (softmax
    denominator columns, output [d,q]->[q,d]) are single-call xbar DMA
    transposes with 3D out APs, all serialized on the sync HWDGE ring
    (concurrent xbar use from two rings corrupts data - minitest3).
  - x inputs arrive host-pre-transposed; all loads are plain DMAs split
    across the scalar + sync rings.
  - E and the tree-sum accumulators are FLAT 2D tiles; the denominator
    tree is one contiguous gpsimd add (L1 of k-tiles 0:8) in parallel
    with DVE adds (k-tiles 8:16 + merge levels).
  - every projection sub-task's PSUM->SBUF consumer (bias-add / V copy)
    is emitted one injection slot AFTER its matmuls so the DVE FIFO
    never head-of-line blocks on un-run PE work.
  - 24-unit software pipeline: unit u emits scores at step u, attn@V at
    step u+1, tail (transposes, divide+bias, store) at step u+2.
"""

import sys
import os

for _p in ("/root/.axon_site/_ro/trn_rl_repo", "/opt/trn_rl_repo"):
    if os.path.isdir(_p) and _p not in sys.path:
        sys.path.append(_p)

import numpy as np
import ml_dtypes

import concourse.bass as bass
import concourse.tile as tile
from concourse import bacc, mybir
from concourse.bass_utils import run_bass_kernel_spmd

B, M, NTOK, DIM = 4, 3, 2048, 512
H, HD = 4, 128
NBM = B * M  # 12
NCORES = 8
SCALE = 1.0 / float(np.sqrt(HD))

F32 = mybir.dt.float32
BF16 = mybir.dt.bfloat16
FP8 = mybir.dt.float8e4
DR = mybir.MatmulPerfMode.DoubleRow
WSCALE = 32.0  # host multiplies Wq/Wk by this before fp8 quantization

TT = NTOK // 128  # 16 k tiles
CT = DIM // 128  # 4 contraction tiles
QCH = 512  # q processed in chunks of 512
NQC = NTOK // QCH  # 4

# Knobs the test harness may flip before calling kernel():
TRACE = False
TRACE_KWARGS = {}
LAST_RESULTS = None

MULT = mybir.AluOpType.mult
ADD = mybir.AluOpType.add
EXP = mybir.ActivationFunctionType.Exp
AXX = mybir.AxisListType.X


def _build_program():
    nc = bacc.Bacc()
    dram = {}
    for s in ("a", "b"):
        D = 512 if s == "a" else 256
        for nm in ("xq", "xk"):
            dram[f"{nm}_{s}"] = nc.dram_tensor(
                f"{nm}_{s}", [DIM, NTOK], FP8, kind="ExternalInput"
            )
        dram[f"xv_{s}"] = nc.dram_tensor(
            f"xv_{s}", [DIM, NTOK], BF16, kind="ExternalInput"
        )
        for nm in ("wq", "wk"):
            dram[f"{nm}_{s}"] = nc.dram_tensor(
                f"{nm}_{s}", [DIM, D], FP8, kind="ExternalInput"
            )
        dram[f"wv_{s}"] = nc.dram_tensor(
            f"wv_{s}", [DIM, D], BF16, kind="ExternalInput"
        )
        for nm in ("bq", "bk"):
            dram[f"{nm}_{s}"] = nc.dram_tensor(
                f"{nm}_{s}", [D], F32, kind="ExternalInput"
            )
        dram[f"bv_{s}"] = nc.dram_tensor(f"bv_{s}", [D], BF16, kind="ExternalInput")
        dram[f"out_{s}"] = nc.dram_tensor(
            f"out_{s}", [NTOK, D], BF16, kind="ExternalOutput"
        )

    with tile.TileContext(nc) as tc:
        with (
            tc.tile_pool(name="wp", bufs=1) as wp,
            tc.tile_pool(name="biasp", bufs=1) as biasp,
            tc.tile_pool(name="xtp", bufs=1) as xtp,
            tc.tile_pool(name="qkvp", bufs=1) as qkvp,
            tc.tile_pool(name="ep", bufs=2) as ep,
            tc.tile_pool(name="accp", bufs=3) as accp,
            tc.tile_pool(name="accvp", bufs=2) as accvp,
            tc.tile_pool(name="accTp", bufs=2) as accTp,
            tc.tile_pool(name="outTp", bufs=2) as outTp,
            tc.tile_pool(name="otqp", bufs=2) as otqp,
            tc.tile_pool(name="otp", bufs=2) as otp,
            tc.tile_pool(name="recp", bufs=2) as recp,
            tc.tile_pool(name="pst", bufs=2, space="PSUM") as pst,
            tc.tile_pool(name="ppv", bufs=2, space="PSUM") as ppv,
        ):
            _emit(nc, dram, wp, biasp, xtp, qkvp, ep, accp, accvp, accTp,
                  outTp, otqp, otp, recp, pst, ppv)

    nc.finalize()
    return nc


def _emit(nc, dram, wp, biasp, xtp, qkvp, ep, accp, accvp, accTp, outTp,
          otqp, otp, recp, pst, ppv):
    # ---------------- prologue loads, split across the two HWDGE rings ----
    # scalar ring: wq, wk, xq_a, xk_a       sync ring: wv, xv_a, biases
    ws = {}

    def load_w(wname, s, eng):
        D = 512 if s == "a" else 256
        dt = BF16 if wname == "wv" else FP8
        w = wp.tile([128, CT, D], dt, tag=f"{wname}_{s}", name=f"{wname}_{s}")
        eng.dma_start(
            out=w[:, :, :],
            in_=dram[f"{wname}_{s}"][:].rearrange("(c p) d -> p c d", p=128),
        )
        ws[f"{wname}_{s}"] = w

    xts = {}  # (slot, kind, ct) -> tile ; tags shared between slots

    def load_x(s, kind, eng):
        xr = dram[f"x{kind}_{s}"][:].rearrange("(c p) M -> p c M", p=128)
        if kind in ("q", "k"):
            xt = xtp.tile([128, CT, NTOK], FP8, tag=f"x{kind}8", name=f"x{kind}8")
            eng.dma_start(out=xt[:, :, :], in_=xr[:, :])
            xts[(s, kind)] = xt
        else:
            for ct in range(CT):
                xt = xtp.tile([128, NTOK], BF16, tag=f"x{kind}{ct}",
                              name=f"x{kind}{ct}")
                eng.dma_start(out=xt[:, :], in_=xr[:, ct])
                xts[(s, kind, ct)] = xt

    bqk = {}
    bvb = {}

    def load_biases(s, D, eng):
        nh = D // 128
        t = biasp.tile([128, 2, nh], F32, tag=f"bqk_{s}", name=f"bqk_{s}")
        eng.dma_start(
            out=t[:, 0, :], in_=dram[f"bq_{s}"][:].rearrange("(j p) -> p j", p=128)
        )
        eng.dma_start(
            out=t[:, 1, :], in_=dram[f"bk_{s}"][:].rearrange("(j p) -> p j", p=128)
        )
        bqk[s] = t
        bv = biasp.tile([128, D], BF16, tag=f"bvb_{s}", name=f"bvb_{s}")
        eng.dma_start(
            out=bv[:, :], in_=dram[f"bv_{s}"][:].unsqueeze(0).to_broadcast([128, D])
        )
        bvb[s] = bv

    # scalar ring: wq, xq, wk (Q-projection critical path), then slot-B w
    # sync ring: biases, xk, wv, xv, slot-B wv/biases
    load_w("wq", "a", nc.scalar)
    load_x("a", "q", nc.scalar)
    load_w("wk", "a", nc.scalar)
    load_biases("a", 512, nc.sync)
    load_x("a", "k", nc.sync)
    load_w("wv", "a", nc.sync)
    load_x("a", "v", nc.sync)
    load_w("wq", "b", nc.scalar)
    load_w("wk", "b", nc.scalar)
    load_w("wv", "b", nc.scalar)
    load_biases("b", 256, nc.scalar)

    QT = {}
    KT = {}
    for s, nh in (("a", 4), ("b", 2)):
        for h in range(nh):
            QT[(s, h)] = qkvp.tile([128, NTOK], BF16, tag=f"qt_{s}{h}",
                                   name=f"qt_{s}{h}")
            KT[(s, h)] = qkvp.tile([128, NTOK], BF16, tag=f"kt_{s}{h}",
                                   name=f"kt_{s}{h}")
    V = {"a": qkvp.tile([128, TT, 512], BF16, tag="v_a", name="v_a"),
         "b": qkvp.tile([128, TT, 256], BF16, tag="v_b", name="v_b")}

    # ---- projection sub-tasks: (emit_mms, emit_post) pairs.  The post
    # (PSUM->SBUF bias-add / copy on DVE) runs one injection slot later so
    # the DVE FIFO never blocks on un-run PE matmuls.
    def qk_sub(s, which, wname, dt, qc4):
        kind = "q" if which == 0 else "k"
        box = {}

        def mms():
            ps = ppv.tile([128, 512], F32, tag="psproj", name="psproj")
            for g2 in range(CT // 2):
                nc.tensor.matmul(
                    ps[:, :],
                    ws[f"{wname}_{s}"][:, 2 * g2 : 2 * g2 + 2,
                                       dt * 128 : (dt + 1) * 128],
                    xts[(s, kind)][:, 2 * g2 : 2 * g2 + 2,
                                   qc4 * 512 : (qc4 + 1) * 512],
                    start=(g2 == 0),
                    stop=(g2 == CT // 2 - 1),
                    perf_mode=DR,
                )
            box["ps"] = ps

        def post():
            dst = QT[(s, dt)] if which == 0 else KT[(s, dt)]
            nc.vector.tensor_scalar(
                dst[:, qc4 * 512 : (qc4 + 1) * 512],
                box["ps"][:, :],
                1.0 / WSCALE,
                bqk[s][:, which, dt : dt + 1],
                MULT,
                ADD,
            )
        return (mms, post)

    def v_sub(s, D, tt):
        box = {}

        def mms():
            ps = ppv.tile([128, 512], F32, tag="psproj", name="psproj")
            for ct in range(CT):
                nc.tensor.matmul(
                    ps[:, :D],
                    xts[(s, "v", ct)][:, tt * 128 : (tt + 1) * 128],
                    ws[f"wv_{s}"][:, ct, :],
                    start=(ct == 0),
                    stop=(ct == CT - 1),
                )
            box["ps"] = ps

        def post():
            nc.scalar.copy(V[s][:, tt, :], box["ps"][:, :D])
        return (mms, post)

    def run_subs(subs):
        # emit a list of (mms, post) with posts deferred by one sub
        pending = None
        for mms, post in subs:
            mms()
            if pending is not None:
                pending()
            pending = post
        if pending is not None:
            pending()

    # slot A h0 Q/K emitted in prologue (before step 0), posts deferred
    run_subs([qk_sub("a", w, ("wq", "wk")[w], 0, qc4)
              for qc4 in range(NQC) for w in range(2)])

    # injection schedule: step -> list of (mms, post) or plain closures
    inject = {st: [] for st in range(26)}

    def add_qk(step0, s, h):
        subs = [qk_sub(s, w, ("wq", "wk")[w], h, qc4)
                for w in range(2) for qc4 in range(NQC)]
        for i, sub in enumerate(subs):  # 8 subs over 2 steps
            inject[step0 + i // 4].append(sub)

    def add_v(step0, s, D, nsteps):
        subs = [v_sub(s, D, tt) for tt in range(TT)]
        per = (len(subs) + nsteps - 1) // nsteps
        for i, sub in enumerate(subs):
            inject[step0 + i // per].append(sub)


    # V_a: 12 token-tiles in step 0, last 4 early in step 1 (their posts
    # land at g-slots 1..7, always ahead of AV_0's matching k-tile reads)
    va_subs = [v_sub("a", 512, tt) for tt in range(TT)]
    for sub in va_subs[0:12]:
        inject[0].append(sub)
    for sub in va_subs[12:16]:
        inject[1].append(sub)
    add_qk(2, "a", 1)            # h1 by end of step 3 (used step 4)
    add_qk(4, "a", 2)            # h2 by end of step 5 (used step 8)
    add_qk(6, "a", 3)            # h3 by end of step 7 (used step 12)
    # slot-B loads are emitted AFTER a step's g-loop ("late") so a DMA
    # issue waiting on an SBUF tag release never head-of-line blocks the
    # issuing engine mid-step.
    late = {1: [("b", "v", nc.sync)],      # xv tags free after step 0
            8: [("b", "q", nc.scalar),     # xq/xk tags free after step 7
                ("b", "k", nc.sync)]}
    add_v(9, "b", 256, 4)        # steps 9-12 (AV_16 runs in step 17)
    add_qk(13, "b", 0)           # by end of step 14 (used step 16)
    add_qk(16, "b", 1)           # by end of step 17 (used step 20)

    # ---------------- the 24-unit software-pipelined stream ---------------
    units = [("a", h, qc) for h in range(4) for qc in range(NQC)] + \
            [("b", h, qc) for h in range(2) for qc in range(NQC)]

    state = {}

    def emit_scores_g(u, g, st_tile):
        s, h, qc = units[u]
        qsl = slice(qc * QCH, (qc + 1) * QCH)
        for j in range(2):
            kt = 2 * g + j
            nc.tensor.matmul(
                st_tile[:, j * QCH : (j + 1) * QCH],
                KT[(s, h)][:, kt * 128 : (kt + 1) * 128],
                QT[(s, h)][:, qsl],
                start=True,
                stop=True,
            )

    def emit_av_g(u, g):
        s, h, qc = units[u]
        stt = state[u]
        for j in range(2):
            kt = 2 * g + j
            nc.tensor.matmul(
                stt["pv"][:, :],
                V[s][:, kt, h * 128 : (h + 1) * 128],
                stt["E"][:, kt * QCH : (kt + 1) * QCH],
                start=(kt == 0),
                stop=(kt == TT - 1),
            )

    NSTEPS = len(units) + 2
    for step in range(NSTEPS):
        u_cur = step if step < len(units) else None
        u_prev = step - 1 if 1 <= step <= len(units) else None
        u_tail = step - 2 if step >= 2 else None

        if u_cur is not None:
            E = ep.tile([128, TT * QCH], BF16, tag="E", name="E")
            state[u_cur] = {"E": E}
        if u_prev is not None:
            pv = ppv.tile([128, QCH], F32, tag="pvav", name="pvav")
            state[u_prev]["pv"] = pv

        inj = list(inject.get(step, ()))
        ninj = len(inj)
        # DVE halves of unit u-1's tree-sum run interleaved in THIS step's
        # g-slots: their E input completed last step, so the DVE FIFO never
        # blocks on them, and they no longer delay this step's proj posts.
        tree_parts = state[u_prev].pop("dve_tree", []) if u_prev is not None else []

        # interleaved PE stream: projections | attn@V(u-1) | scores(u).
        # Projection posts (DVE) are deferred one g-slot behind their MMs.
        pending_posts = []
        per_slot = -(-ninj // 8)  # ceil: front-load so posts land early
        for g in range(8):
            lo = min(g * per_slot, ninj)
            hi = min((g + 1) * per_slot, ninj)
            posts_due, pending_posts = pending_posts, []
            for mms, post in inj[lo:hi]:
                mms()
                if post is not None:
                    pending_posts.append(post)
            for p in posts_due:
                p()
            if 2 <= g < 2 + len(tree_parts):
                tree_parts[g - 2]()
            if u_prev is not None:
                emit_av_g(u_prev, g)
            if u_cur is not None:
                st_tile = pst.tile([128, 2 * QCH], F32, tag="st", name="st")
                emit_scores_g(u_cur, g, st_tile)
                nc.scalar.activation(
                    state[u_cur]["E"][:, g * 1024 : (g + 1) * 1024],
                    st_tile[:, :],
                    EXP,
                    scale=SCALE,
                )
        for p in pending_posts:
            p()
        for (ls, lk, leng) in late.get(step, ()):
            load_x(ls, lk, leng)

        # tail of unit u-2
        if u_tail is not None:
            ts, th, tqc = units[u_tail]
            stt = state[u_tail]
            accT = accTp.tile([128, NQC, 128], BF16, tag="accT", name="accT")
            nc.sync.dma_start_transpose(out=accT[:, :, :], in_=stt["acc"][:, 0:512])
            otq = otqp.tile([128, NQC, 128], BF16, tag="otq", name="otq")
            nc.sync.dma_start_transpose(out=otq[:, :, :], in_=stt["outT"][:, :])
            dcol4 = recp.tile([128, NQC], F32, tag="dcol4", name="dcol4")
            rec4 = recp.tile([128, NQC], BF16, tag="rec4", name="rec4")
            nc.vector.reduce_sum(out=dcol4[:, :], in_=accT[:, :, :], axis=AXX)
            with nc.allow_low_precision(reason="softmax denom reciprocal in bf16"):
                nc.vector.reciprocal(rec4[:, :], dcol4[:, :])
            ot = otp.tile([128, NQC, 128], BF16, tag="ot", name="ot")
            for j in range(NQC):
                eng = nc.vector
                eng.scalar_tensor_tensor(
                    out=ot[:, j, :],
                    in0=otq[:, j, :],
                    scalar=rec4[:, j : j + 1],
                    in1=bvb[ts][:, th * 128 : (th + 1) * 128],
                    op0=MULT,
                    op1=ADD,
                )
            nc.sync.dma_start(
                out=dram[f"out_{ts}"][
                    tqc * QCH : (tqc + 1) * QCH, th * 128 : (th + 1) * 128
                ].rearrange("(j p) d -> p j d", p=128),
                in_=ot[:, :, :],
            )
            del state[u_tail]

        # cast of pv(u-1) -> bf16 SBUF for the xbar transpose next step
        if u_prev is not None:
            outT = outTp.tile([128, QCH], BF16, tag="outT", name="outT")
            nc.vector.tensor_copy(outT[:, :], state[u_prev]["pv"][:, :])
            state[u_prev]["outT"] = outT

        # tree-sum of unit u: all-DVE (gpsimd shares the DVE SBUF ports, so
        # offloading there just steals DVE bandwidth), emitted as closures
        # run in step u+1's g-slot interleave where E is already complete.
        if u_cur is not None:
            E = state[u_cur]["E"]
            acc_g = accp.tile([128, 4096], BF16, tag="acc_g", name="acc_g")

            def t_l1(E=E, acc_g=acc_g):
                nc.vector.tensor_add(acc_g[:, :], E[:, 0:4096], E[:, 4096:8192])

            def t_l2(acc_g=acc_g):
                nc.vector.tensor_add(
                    acc_g[:, 0:2048], acc_g[:, 0:2048], acc_g[:, 2048:4096]
                )

            def t_l3(acc_g=acc_g):
                nc.vector.tensor_add(
                    acc_g[:, 0:1024], acc_g[:, 0:1024], acc_g[:, 1024:2048]
                )

            def t_l4(acc_g=acc_g):
                nc.vector.tensor_add(
                    acc_g[:, 0:512], acc_g[:, 0:512], acc_g[:, 512:1024]
                )

            state[u_cur]["dve_tree"] = [t_l1, t_l2, t_l3, t_l4]
            state[u_cur]["acc"] = acc_g


_PROGRAM = None


def _get_program():
    global _PROGRAM
    if _PROGRAM is None:
        _PROGRAM = _build_program()
    return _PROGRAM


def kernel(query, key, value, Wq, bq, Wk, bk, Wv, bv):
    global LAST_RESULTS
    bf = ml_dtypes.bfloat16
    f8 = ml_dtypes.float8_e4m3fn
    q = np.ascontiguousarray(
        np.asarray(query, np.float32).reshape(NBM, NTOK, DIM).transpose(0, 2, 1)
    ).astype(f8)
    k = np.ascontiguousarray(
        np.asarray(key, np.float32).reshape(NBM, NTOK, DIM).transpose(0, 2, 1)
    ).astype(f8)
    v = np.ascontiguousarray(
        np.asarray(value, np.float32).reshape(NBM, NTOK, DIM).transpose(0, 2, 1)
    ).astype(bf)
    WqT = np.ascontiguousarray(np.asarray(Wq, np.float32).T * WSCALE).astype(f8)
    WkT = np.ascontiguousarray(np.asarray(Wk, np.float32).T * WSCALE).astype(f8)
    WvT = np.ascontiguousarray(np.asarray(Wv, np.float32).T).astype(bf)
    bq = np.asarray(bq, np.float32)
    bk = np.asarray(bk, np.float32)
    bvb = np.asarray(bv, np.float32).astype(bf)

    in_maps = []
    for c in range(NCORES):
        bm_a = c
        bm_b = 8 + c // 2
        hs = (c % 2) * 256  # head-pair column offset for slot B
        in_maps.append(
            {
                "xq_a": q[bm_a], "xk_a": k[bm_a], "xv_a": v[bm_a],
                "xq_b": q[bm_b], "xk_b": k[bm_b], "xv_b": v[bm_b],
                "wq_a": WqT, "wk_a": WkT, "wv_a": WvT,
                "bq_a": bq, "bk_a": bk, "bv_a": bvb,
                "wq_b": np.ascontiguousarray(WqT[:, hs : hs + 256]),
                "wk_b": np.ascontiguousarray(WkT[:, hs : hs + 256]),
                "wv_b": np.ascontiguousarray(WvT[:, hs : hs + 256]),
                "bq_b": np.ascontiguousarray(bq[hs : hs + 256]),
                "bk_b": np.ascontiguousarray(bk[hs : hs + 256]),
                "bv_b": np.ascontiguousarray(bvb[hs : hs + 256]),
            }
        )

    nc = _get_program()
    res = run_bass_kernel_spmd(
        nc, in_maps, list(range(NCORES)), trace=TRACE, **TRACE_KWARGS
    )
    LAST_RESULTS = res

    out = np.empty((NBM, NTOK, DIM), np.float32)
    for c in range(NCORES):
        hs = (c % 2) * 256
        out[c] = np.asarray(res.results[c]["out_a"], np.float32)
        out[8 + c // 2][:, hs : hs + 256] = np.asarray(
            res.results[c]["out_b"], np.float32
        )
    return out.reshape(B, M, NTOK, DIM)
